# revision 37
# baseline (speedup 1.0000x reference)
"""Entmax (alpha=1.25) bisection kernel for Trainium2, 8 NeuronCores.

Solves  sum_j relu(x_j/4 - t)^4 = 1  per row (t = tau'/4) and emits the
normalized probabilities  p = relu(x/4 - t)^4  (Z == 1 by construction).

Per 128-row tile:
  1. stream the tile in, pre-scale to fp16 x16 = fp16(x/4) (DVE tensor_scalar,
     2x mode), and build an 8:1 max tree per 1600-chunk -> lmax[4000];
     grouped reduce -> cmax[500].
  2. bisect G(t) = sum relu(cmax - t)^4 >= 1 for 5 iters on fp16 cmax: a
     certified lower bound theta <= t* (G <= F pointwise for any grouping).
  3. masked power sums at theta: ym = max(x16, theta) (DVE tensor_scalar,
     sum-accum -> A1 + W*theta), z = Square(ym - theta) (ACT, bias, accum A2),
     y^3 = z*ym on Pool (in place), summed by a DVE copy-accum
     (-> A3 + theta*A2), w = Square(z) (ACT, accum A4, dump buffer).
  4. Newton on P(d) = A4-1 - 4A3 d + 6A2 d^2 - 4A1 d^3 = 0 via fused
     scalar_tensor_tensor Horner steps  ->  tau = theta + d.
  5. output: yo = relu(x16 - tau) (DVE 4x), zo = yo^2 (DVE 2x),
     out = zo^2 -> f32 (ACT Square / DVE tensor_tensor), chunk-wise DMA out.

Both row tiles' fp16 copies fit in SBUF simultaneously, so tile 1 streams in
while tile 0 computes.  Engines execute their queues nearly in order, so
emission interleaves the two tiles' phases (tile-1 loads inside tile-0's
moment loop, tile-1 moments inside tile-0's output loop) to keep DVE, ACT,
and Pool all busy.  Loads alternate between the two staging pools to deepen
DMA double-buffering at no SBUF cost.
"""

import numpy as np

import concourse.mybir as mybir
from concourse.tile import TileContext

P = 128
D = 32000
ROWS_PER_CORE = 256
N_ROW_TILES = 2
N_CORES = 8

CHUNK = 1600
N_CHUNKS = D // CHUNK            # 20
SC = 3200                        # super-chunk for moment/output passes
N_SC = D // SC                   # 10
LM_W = N_CHUNKS * (CHUNK // 8)   # 4000  (8:1 tree per chunk)
CM_W = LM_W // 8                 # 500   (grouped reduce b=8)

G_ITERS = 5
G_MARGIN = 0.004                 # prescaled units
NEWTON_ITERS = 3

F32 = mybir.dt.float32
F16 = mybir.dt.float16
DM0 = float(np.float32(1.0 - (1.0 / D) ** 0.25))   # prescaled bracket width


def _emit_load_chunk(nc, pools, x16, c, x_dram, row0, m1_pool=True):
    """DMA one chunk into alternating staging, pre-scale to fp16, and run the
    8:1 max tree.  m1 runs on Pool during tile-0's load phase (Pool is idle
    there) and on DVE during tile-0's moment phase (Pool runs the A3 muls)."""
    Alu = mybir.AluOpType
    sl = slice(c * CHUNK, (c + 1) * CHUNK)
    n_ld = pools["ldctr"][0]
    pools["ldctr"][0] += 1
    stp = pools["ist"] if n_ld % 2 == 0 else pools["ost"]
    st = stp.tile([P, CHUNK], F32, tag="st", name="st")
    # tile 0 spreads loads over three DMA queues (per-queue issue rate is
    # ~4.5us; three queues let the last chunk land at DMA-bandwidth pace).
    # tile 1 uses two queues with flipped parity so each half queues behind
    # tile 0's in the SAME engine queue (no overtaking of tile 0's tail).
    if c >= N_CHUNKS - 4:
        # the tile's last chunks gate the bracket: keep them on the SP queue
        # so per-queue issue pacing cannot delay the tree/bisect start
        ldq = nc.sync
    elif pools.get("qflip"):
        ldq = nc.scalar if n_ld % 2 == 0 else nc.sync
    else:
        ldq = nc.sync if n_ld % 2 == 0 else nc.scalar
    ldq.dma_start(out=st, in_=x_dram[row0 : row0 + P, sl])
    qs = pools.get("qscale")
    nc.vector.tensor_scalar(x16[:, sl], st, qs if qs is not None else 0.25,
                            None, op0=Alu.mult)
    h1, h2, h3 = CHUNK // 2, CHUNK // 4, CHUNK // 8
    m1 = pools["m1"].tile([P, h1], F16, tag="m1", name="m1")
    nc.vector.tensor_max(m1, x16[:, c * CHUNK : c * CHUNK + h1],
                         x16[:, c * CHUNK + h1 : (c + 1) * CHUNK])
    m2 = pools["m2"].tile([P, h2], F16, tag="m2", name="m2")
    nc.vector.tensor_max(m2, m1[:, :h2], m1[:, h2:])
    lm = pools["lmax_t"]
    nc.vector.tensor_max(lm[:, c * h3 : (c + 1) * h3], m2[:, :h3], m2[:, h3:])


def _emit_bracket(nc, pools, t):
    """Grouped reduce -> cmax, then 5-iter bisection -> theta (all DVE)."""
    Alu = mybir.AluOpType
    sm = pools["small"]
    lm = pools["lmax_t"]
    cmax = sm.tile([P, CM_W], F16, tag=f"cmax{t}", name="cmax")
    nc.vector.reduce_max(
        out=cmax, in_=lm.rearrange("p (a b) -> p a b", b=8),
        axis=mybir.AxisListType.X,
    )
    rmax = sm.tile([P, 1], F16, tag=f"rmax{t}", name="rmax")
    nc.vector.reduce_max(out=rmax, in_=cmax, axis=mybir.AxisListType.X)
    lo = sm.tile([P, 1], F32, tag=f"lo{t}", name="lo")
    nc.vector.tensor_scalar(lo, rmax, 1.0, None, op0=Alu.subtract)

    tm = sm.tile([P, 1], F32, tag=f"tm{t}", name="tm")
    gv = sm.tile([P, 1], F32, tag=f"gv{t}", name="gv")
    ind = sm.tile([P, 1], F32, tag=f"ind{t}", name="ind")
    bp = pools["bis"]
    for i in range(G_ITERS):
        dm_i = float(np.float32(DM0 * 0.5 ** (i + 1)))
        nc.vector.tensor_scalar(tm, lo, dm_i, None, op0=Alu.add)
        yg = bp.tile([P, CM_W], F16, tag="yg", name="yg")
        nc.vector.tensor_scalar(yg, cmax, tm, 0.0, op0=Alu.subtract, op1=Alu.max)
        zg = bp.tile([P, CM_W], F16, tag="zg", name="zg")
        nc.vector.tensor_mul(zg, yg, yg)
        wg = bp.tile([P, CM_W], F16, tag="wg", name="wg")
        nc.vector.scalar_tensor_tensor(
            out=wg, in0=zg, scalar=1.0, in1=zg, op0=Alu.mult, op1=Alu.mult,
            accum_out=gv,
        )
        nc.vector.tensor_scalar(ind, gv, 1.0, None, op0=Alu.is_ge)
        nc.vector.scalar_tensor_tensor(
            out=lo, in0=ind, scalar=dm_i, in1=lo, op0=Alu.mult, op1=Alu.add,
        )
    theta = sm.tile([P, 1], F32, tag=f"theta{t}", name="theta")
    nc.vector.tensor_scalar(theta, lo, -G_MARGIN, None, op0=Alu.add)
    ntheta = sm.tile([P, 1], F32, tag=f"ntheta{t}", name="ntheta")
    nc.vector.tensor_scalar(ntheta, theta, -1.0, None, op0=Alu.mult)
    pools[f"cmax_tile{t}"] = cmax
    return theta, ntheta


def _emit_moment_sc(nc, pools, t, s, x16, theta, ntheta):
    """One super-chunk of the moment pass.
    ym = max(x16, theta) (DVE, accum -> A1 + W*theta)
    z  = Square(ym - theta) (ACT bias, accum A2)
    ym <- z*ym  (Pool, in place)         [= y^3]
    A3 partial: DVE copy-accum of ym     [= A3 + theta*A2]
    w  = Square(z) -> lmax dump (ACT, accum A4)"""
    Alu = mybir.AluOpType
    Act = mybir.ActivationFunctionType
    sl = slice(s * SC, (s + 1) * SC)
    a1p, a2p, a3p, a4p = pools[f"aparts{t}"]
    ym = pools["y"].tile([P, SC], F16, tag="y", name="ym")
    nc.vector.tensor_scalar(
        ym, x16[:, sl], theta, None, op0=Alu.max, op1=Alu.add,
        accum_out=a1p[:, s : s + 1],
    )
    z = pools["z"].tile([P, SC], F16, tag="z", name="z")
    nc.scalar.activation(
        z, ym, Act.Square, bias=ntheta, scale=1.0, accum_out=a2p[:, s : s + 1]
    )
    nc.gpsimd.tensor_mul(ym, z, ym)
    nc.vector.tensor_scalar(
        ym, ym, 1.0, 0.0, op0=Alu.mult, op1=Alu.add, accum_out=a3p[:, s : s + 1]
    )
    nc.scalar.activation(
        pools["lmax_t"][:, :SC], z, Act.Square, accum_out=a4p[:, s : s + 1]
    )


def _emit_newton(nc, pools, t, theta):
    """Reduce moment partials (with theta-shift corrections), then Newton on
    P(d) = a4m + k1 d + k2 d^2 + k3 d^3 using n = -d and fused STT Horner:
      P  = ((mk3*n + k2)*n - (-mk1))*n + a4m  with mk1 = -k1, mk3 = -k3
      P' = (q3*n + mq2)*n - mk1               with q3 = 3k3, mq2 = -2k2
      n <- n + P/P'
    Returns tau = theta - n."""
    Alu = mybir.AluOpType
    sm = pools["small"]
    a1p, a2p, a3p, a4p = pools[f"aparts{t}"]

    def tile(nm):
        return sm.tile([P, 1], F32, tag=f"{nm}_{t}", name=nm)

    a1s, a2s, a3s, a4s = tile("a1s"), tile("a2s"), tile("a3s"), tile("a4s")
    for acc, prt in zip((a1s, a2s, a3s, a4s), (a1p, a2p, a3p, a4p)):
        nc.vector.reduce_sum(out=acc, in_=prt, axis=mybir.AxisListType.X)

    a4m = tile("a4m")
    nc.vector.tensor_scalar(a4m, a4s, -1.0, None, op0=Alu.add)
    k2 = tile("k2")
    nc.vector.tensor_scalar(k2, a2s, 6.0, None, op0=Alu.mult)
    mq2 = tile("mq2")
    nc.vector.tensor_scalar(mq2, a2s, -12.0, None, op0=Alu.mult)
    # mk3 = 4*A1 = 4*a1s - 4D*theta ; q3 = -12*A1
    t4 = tile("t4")
    nc.vector.tensor_scalar(t4, a1s, 4.0, None, op0=Alu.mult)
    mk3 = tile("mk3")
    nc.vector.scalar_tensor_tensor(
        out=mk3, in0=theta, scalar=float(-4.0 * D), in1=t4,
        op0=Alu.mult, op1=Alu.add,
    )
    q3 = tile("q3")
    nc.vector.tensor_scalar(q3, mk3, -3.0, None, op0=Alu.mult)
    # mk1 = 4*A3 = 4*a3s - 4*theta*A2
    v = tile("v")
    nc.vector.tensor_mul(v, a2s, theta)
    t5 = tile("t5")
    nc.vector.tensor_scalar(t5, a3s, 4.0, None, op0=Alu.mult)
    mk1 = tile("mk1")
    nc.vector.scalar_tensor_tensor(
        out=mk1, in0=v, scalar=-4.0, in1=t5, op0=Alu.mult, op1=Alu.add,
    )

    n = tile("n")
    nc.vector.memset(n, 0.0)
    pv = tile("pv")
    ppv = tile("ppv")
    for _ in range(NEWTON_ITERS):
        nc.vector.scalar_tensor_tensor(
            out=pv, in0=n, scalar=mk3, in1=k2, op0=Alu.mult, op1=Alu.add)
        nc.vector.scalar_tensor_tensor(
            out=pv, in0=pv, scalar=n, in1=mk1, op0=Alu.mult, op1=Alu.add)
        nc.vector.scalar_tensor_tensor(
            out=pv, in0=pv, scalar=n, in1=a4m, op0=Alu.mult, op1=Alu.add)
        nc.vector.scalar_tensor_tensor(
            out=ppv, in0=n, scalar=q3, in1=mq2, op0=Alu.mult, op1=Alu.add)
        nc.vector.scalar_tensor_tensor(
            out=ppv, in0=ppv, scalar=n, in1=mk1, op0=Alu.mult, op1=Alu.subtract)
        nc.vector.reciprocal(ppv, ppv)
        nc.vector.scalar_tensor_tensor(
            out=n, in0=pv, scalar=ppv, in1=n, op0=Alu.mult, op1=Alu.add)

    tau = sm.tile([P, 1], F32, tag=f"tau{t}", name="tau")
    nc.vector.scalar_tensor_tensor(
        out=tau, in0=n, scalar=-1.0, in1=theta, op0=Alu.mult, op1=Alu.add)
    return tau


def _emit_output_sc(nc, pools, s, x16, tau, out_dram, row0, out0_dve):
    """One super-chunk of the output pass: yo (DVE 4x), zo (DVE 2x), then the
    two f32 out chunks: chunk 0 on DVE tensor_tensor when out0_dve (to
    offload ACT in the merged phase), chunk 1 on ACT Square."""
    Alu = mybir.AluOpType
    Act = mybir.ActivationFunctionType
    sl = slice(s * SC, (s + 1) * SC)
    yo = pools["y"].tile([P, SC], F16, tag="y", name="yo")
    nc.vector.tensor_scalar(yo, x16[:, sl], tau, 0.0, op0=Alu.subtract, op1=Alu.max)
    zo = pools["z"].tile([P, SC], F16, tag="z", name="zo")
    nc.vector.tensor_mul(zo, yo, yo)
    for k in range(SC // CHUNK):
        c = s * (SC // CHUNK) + k
        # loads are done by the time outputs flow: alternate across BOTH
        # staging pools for 4-deep rotation (store-sem latency hiding)
        n_o = pools["ldctr"][0]
        pools["ldctr"][0] += 1
        ostp = pools["ost"] if n_o % 2 == 0 else pools["ist"]
        ost = ostp.tile([P, CHUNK], F32, tag="st", name="ost")
        zsl = zo[:, k * CHUNK : (k + 1) * CHUNK]
        if k == 0 and out0_dve:
            nc.vector.tensor_mul(ost, zsl, zsl)
        else:
            nc.scalar.activation(ost, zsl, Act.Square)
        n_st = pools["stctr"][0]
        pools["stctr"][0] += 1
        stq = nc.sync if n_st % 2 == 0 else nc.scalar
        stq.dma_start(
            out=out_dram[row0 : row0 + P, c * CHUNK : (c + 1) * CHUNK], in_=ost
        )


def _core_program(nc, tc, x_dram, out_dram):
    import contextlib

    with contextlib.ExitStack() as stack:
        pools = {}
        for nm, bufs in (("ist", 2), ("ost", 2), ("x16p", 2), ("lmp", 2),
                         ("m1", 1), ("m2", 1), ("y", 2), ("z", 2),
                         ("bis", 1), ("small", 1)):
            pools[nm] = stack.enter_context(tc.tile_pool(name=nm, bufs=bufs))
        pools["ldctr"] = [0]
        pools["stctr"] = [0]
        sm = pools["small"]

        def tile_pools(t):
            d = dict(pools)
            d[f"aparts{t}"] = tuple(
                sm.tile([P, N_SC], F32, tag=f"a{m}p{t}", name=f"a{m}p{t}")
                for m in range(1, 5)
            )
            return d

        row0 = [t * P for t in range(N_ROW_TILES)]
        x16 = [pools["x16p"].tile([P, D], F16, tag="x16", name="x16")
               for _ in range(2)]
        tp = [tile_pools(0), tile_pools(1)]

        # ---- phase A0: tile 0 loads + tree (m1 on Pool) -------------------
        tp[0]["lmax_t"] = pools["lmp"].tile([P, LM_W], F16, tag="lmax",
                                            name="lmax")
        for c in range(N_CHUNKS):
            _emit_load_chunk(nc, tp[0], x16[0], c, x_dram, row0[0],
                             m1_pool=True)

        # ---- phase B0: tile 0 bracket + bisect ----------------------------
        # high_priority: the bracket gates everything downstream; without it
        # the Tile scheduler interleaves tile-1 conv ops (whose loads land
        # ~35us later) ahead of the reduce in the DVE queue.
        with tc.high_priority():
            theta0, ntheta0 = _emit_bracket(nc, tp[0], 0)
        # scale constant written AFTER tile 0's bracket: tile 1's conversions
        # read it, which forces the scheduler's DAG to place them behind the
        # bracket in the DVE queue (otherwise it interleaves them ahead of
        # the reduce and the bracket stalls on tile 1's late loads)
        qscale = pools["small"].tile([P, 1], F32, tag="qscale", name="qscale")
        # derive the constant from cmax so the dependency is real: 0*cmax+0.25
        nc.vector.tensor_scalar(
            qscale, tp[0]["cmax_tile0"][:, 0:1],
            0.0, 0.25, op0=mybir.AluOpType.mult, op1=mybir.AluOpType.add)
        tp[1]["qscale"] = qscale

        # ---- phase C0: tile 0 moments + tile 1 loads (m1 on DVE) ----------
        tp[1]["lmax_t"] = pools["lmp"].tile([P, LM_W], F16, tag="lmax",
                                            name="lmax")
        tp[1]["qflip"] = True
        for s in range(N_SC):
            _emit_moment_sc(nc, tp[0], 0, s, x16[0], theta0, ntheta0)
            with tc.high_priority(offset=-2000):
                for k in range(2):
                    _emit_load_chunk(nc, tp[1], x16[1], 2 * s + k, x_dram,
                                     row0[1], m1_pool=False)

        # ---- phase B1: tile 1 bracket + bisect (t1 data lands first) ------
        with tc.high_priority(offset=200):
            theta1, ntheta1 = _emit_bracket(nc, tp[1], 1)
        for k in range(2):
            _emit_moment_sc(nc, tp[1], 1, k, x16[1], theta1, ntheta1)

        # ---- phase D0: tile 0 Newton --------------------------------------
        with tc.high_priority(offset=200):
            tau0 = _emit_newton(nc, tp[0], 0, theta0)

        # ---- merged: rest of tile 1 moments + tile 0 output ---------------
        for k in range(N_SC):
            if k + 2 < N_SC:
                _emit_moment_sc(nc, tp[1], 1, k + 2, x16[1], theta1, ntheta1)
            _emit_output_sc(nc, tp[0], k, x16[0], tau0, out_dram,
                            row0[0], out0_dve=True)

        # ---- phase D1: tile 1 Newton --------------------------------------
        tau1 = _emit_newton(nc, tp[1], 1, theta1)

        # ---- phase E1: tile 1 output --------------------------------------
        for s in range(N_SC):
            _emit_output_sc(nc, tp[1], s, x16[1], tau1, out_dram, row0[1],
                            out0_dve=True)


def build_bass():
    from concourse import bacc

    nc = bacc.Bacc(None, target_bir_lowering=False)
    x_dram = nc.dram_tensor("x", [ROWS_PER_CORE, D], F32, kind="ExternalInput")
    out_dram = nc.dram_tensor("out", [ROWS_PER_CORE, D], F32, kind="ExternalOutput")
    with TileContext(nc) as tc:
        _core_program(nc, tc, x_dram, out_dram)
    nc.compile()
    return nc


_NC_CACHE = None


def kernel(input: np.ndarray) -> np.ndarray:
    global _NC_CACHE
    from concourse.bass_utils import run_bass_kernel_spmd

    x = np.ascontiguousarray(input, dtype=np.float32)
    assert x.shape == (ROWS_PER_CORE * N_CORES, D)

    if _NC_CACHE is None:
        _NC_CACHE = build_bass()
    nc = _NC_CACHE

    in_maps = [
        {"x": x[i * ROWS_PER_CORE : (i + 1) * ROWS_PER_CORE]} for i in range(N_CORES)
    ]
    res = run_bass_kernel_spmd(nc, in_maps, core_ids=list(range(N_CORES)))
    return np.concatenate([r["out"] for r in res.results], axis=0)


# revision 40
# speedup vs baseline: 1.0114x; 1.0114x over previous
"""Entmax (alpha=1.25) bisection kernel for Trainium2, 8 NeuronCores.

Solves  sum_j relu(x_j/4 - t)^4 = 1  per row (t = tau'/4) and emits the
normalized probabilities  p = relu(x/4 - t)^4  (Z == 1 by construction).

Per 128-row tile:
  1. stream the tile in, pre-scale to fp16 x16 = fp16(x/4) (DVE tensor_scalar,
     2x mode), and build an 8:1 max tree per 1600-chunk -> lmax[4000];
     grouped reduce -> cmax[500].
  2. bisect G(t) = sum relu(cmax - t)^4 >= 1 for 5 iters on fp16 cmax: a
     certified lower bound theta <= t* (G <= F pointwise for any grouping).
  3. masked power sums at theta: ym = max(x16, theta) (DVE tensor_scalar,
     sum-accum -> A1 + W*theta), z = Square(ym - theta) (ACT, bias, accum A2),
     y^3 = z*ym on Pool (in place), summed by a DVE copy-accum
     (-> A3 + theta*A2), w = Square(z) (ACT, accum A4, dump buffer).
  4. Newton on P(d) = A4-1 - 4A3 d + 6A2 d^2 - 4A1 d^3 = 0 via fused
     scalar_tensor_tensor Horner steps  ->  tau = theta + d.
  5. output: yo = relu(x16 - tau) (DVE 4x), zo = yo^2 (DVE 2x),
     out = zo^2 -> f32 (ACT Square / DVE tensor_tensor), chunk-wise DMA out.

Both row tiles' fp16 copies fit in SBUF simultaneously, so tile 1 streams in
while tile 0 computes.  Engines execute their queues nearly in order, so
emission interleaves the two tiles' phases (tile-1 loads inside tile-0's
moment loop, tile-1 moments inside tile-0's output loop) to keep DVE, ACT,
and Pool all busy.  Loads alternate between the two staging pools to deepen
DMA double-buffering at no SBUF cost.
"""

import numpy as np

import concourse.mybir as mybir
from concourse.tile import TileContext

P = 128
D = 32000
ROWS_PER_CORE = 256
N_ROW_TILES = 2
N_CORES = 8

CHUNK = 1600
N_CHUNKS = D // CHUNK            # 20
SC = 3200                        # super-chunk for moment/output passes
N_SC = D // SC                   # 10
LM_W = N_CHUNKS * (CHUNK // 8)   # 4000  (8:1 tree per chunk)
CM_W = LM_W // 8                 # 500   (grouped reduce b=8)

G_ITERS = 4
G_MARGIN = 0.004                 # prescaled units
NEWTON_ITERS = 3

F32 = mybir.dt.float32
F16 = mybir.dt.float16
DM0 = float(np.float32(1.0 - (1.0 / D) ** 0.25))   # prescaled bracket width


def _emit_load_chunk(nc, pools, x16, c, x_dram, row0, m1_pool=True):
    """DMA one chunk into alternating staging, pre-scale to fp16, and run the
    8:1 max tree.  m1 runs on Pool during tile-0's load phase (Pool is idle
    there) and on DVE during tile-0's moment phase (Pool runs the A3 muls)."""
    Alu = mybir.AluOpType
    sl = slice(c * CHUNK, (c + 1) * CHUNK)
    n_ld = pools["ldctr"][0]
    pools["ldctr"][0] += 1
    stp = pools["ist"] if n_ld % 2 == 0 else pools["ost"]
    st = stp.tile([P, CHUNK], F32, tag="st", name="st")
    # tile 0 spreads loads over three DMA queues (per-queue issue rate is
    # ~4.5us; three queues let the last chunk land at DMA-bandwidth pace).
    # tile 1 uses two queues with flipped parity so each half queues behind
    # tile 0's in the SAME engine queue (no overtaking of tile 0's tail).
    if c >= N_CHUNKS - 4:
        # the tile's last chunks gate the bracket: keep them on the SP queue
        # so per-queue issue pacing cannot delay the tree/bisect start
        ldq = nc.sync
    elif pools.get("qflip"):
        ldq = nc.scalar if n_ld % 2 == 0 else nc.sync
    else:
        ldq = nc.sync if n_ld % 2 == 0 else nc.scalar
    ldq.dma_start(out=st, in_=x_dram[row0 : row0 + P, sl])
    qs = pools.get("qscale")
    nc.vector.tensor_scalar(x16[:, sl], st, qs if qs is not None else 0.25,
                            None, op0=Alu.mult)
    h1, h2, h3 = CHUNK // 2, CHUNK // 4, CHUNK // 8
    m1 = pools["m1"].tile([P, h1], F16, tag="m1", name="m1")
    nc.vector.tensor_max(m1, x16[:, c * CHUNK : c * CHUNK + h1],
                         x16[:, c * CHUNK + h1 : (c + 1) * CHUNK])
    m2 = pools["m2"].tile([P, h2], F16, tag="m2", name="m2")
    nc.vector.tensor_max(m2, m1[:, :h2], m1[:, h2:])
    lm = pools["lmax_t"]
    nc.vector.tensor_max(lm[:, c * h3 : (c + 1) * h3], m2[:, :h3], m2[:, h3:])


def _emit_bracket(nc, pools, t):
    """Grouped reduce -> cmax, then 5-iter bisection -> theta (all DVE)."""
    Alu = mybir.AluOpType
    sm = pools["small"]
    lm = pools["lmax_t"]
    cmax = sm.tile([P, CM_W], F16, tag=f"cmax{t}", name="cmax")
    nc.vector.reduce_max(
        out=cmax, in_=lm.rearrange("p (a b) -> p a b", b=8),
        axis=mybir.AxisListType.X,
    )
    rmax = sm.tile([P, 1], F16, tag=f"rmax{t}", name="rmax")
    nc.vector.reduce_max(out=rmax, in_=cmax, axis=mybir.AxisListType.X)
    lo = sm.tile([P, 1], F32, tag=f"lo{t}", name="lo")
    nc.vector.tensor_scalar(lo, rmax, 1.0, None, op0=Alu.subtract)

    tm = sm.tile([P, 1], F32, tag=f"tm{t}", name="tm")
    gv = sm.tile([P, 1], F32, tag=f"gv{t}", name="gv")
    ind = sm.tile([P, 1], F32, tag=f"ind{t}", name="ind")
    bp = pools["bis"]
    for i in range(G_ITERS):
        dm_i = float(np.float32(DM0 * 0.5 ** (i + 1)))
        nc.vector.tensor_scalar(tm, lo, dm_i, None, op0=Alu.add)
        yg = bp.tile([P, CM_W], F16, tag="yg", name="yg")
        nc.vector.tensor_scalar(yg, cmax, tm, 0.0, op0=Alu.subtract, op1=Alu.max)
        zg = bp.tile([P, CM_W], F16, tag="zg", name="zg")
        nc.vector.tensor_mul(zg, yg, yg)
        wg = bp.tile([P, CM_W], F16, tag="wg", name="wg")
        nc.vector.scalar_tensor_tensor(
            out=wg, in0=zg, scalar=1.0, in1=zg, op0=Alu.mult, op1=Alu.mult,
            accum_out=gv,
        )
        nc.vector.tensor_scalar(ind, gv, 1.0, None, op0=Alu.is_ge)
        nc.vector.scalar_tensor_tensor(
            out=lo, in0=ind, scalar=dm_i, in1=lo, op0=Alu.mult, op1=Alu.add,
        )
    theta = sm.tile([P, 1], F32, tag=f"theta{t}", name="theta")
    nc.vector.tensor_scalar(theta, lo, -G_MARGIN, None, op0=Alu.add)
    ntheta = sm.tile([P, 1], F32, tag=f"ntheta{t}", name="ntheta")
    nc.vector.tensor_scalar(ntheta, theta, -1.0, None, op0=Alu.mult)
    pools[f"cmax_tile{t}"] = cmax
    return theta, ntheta


def _emit_moment_sc(nc, pools, t, s, x16, theta, ntheta):
    """One super-chunk of the moment pass.
    ym = max(x16, theta) (DVE, accum -> A1 + W*theta)
    z  = Square(ym - theta) (ACT bias, accum A2)
    ym <- z*ym  (Pool, in place)         [= y^3]
    A3 partial: DVE copy-accum of ym     [= A3 + theta*A2]
    w  = Square(z) -> lmax dump (ACT, accum A4)"""
    Alu = mybir.AluOpType
    Act = mybir.ActivationFunctionType
    sl = slice(s * SC, (s + 1) * SC)
    a1p, a2p, a3p, a4p = pools[f"aparts{t}"]
    ym = pools["y"].tile([P, SC], F16, tag="y", name="ym")
    nc.vector.tensor_scalar(
        ym, x16[:, sl], theta, None, op0=Alu.max, op1=Alu.add,
        accum_out=a1p[:, s : s + 1],
    )
    z = pools["z"].tile([P, SC], F16, tag="z", name="z")
    nc.scalar.activation(
        z, ym, Act.Square, bias=ntheta, scale=1.0, accum_out=a2p[:, s : s + 1]
    )
    nc.gpsimd.tensor_mul(ym, z, ym)
    nc.vector.tensor_scalar(
        ym, ym, 1.0, 0.0, op0=Alu.mult, op1=Alu.add, accum_out=a3p[:, s : s + 1]
    )
    nc.scalar.activation(
        pools["lmax_t"][:, :SC], z, Act.Square, accum_out=a4p[:, s : s + 1]
    )


def _emit_newton(nc, pools, t, theta):
    """Reduce moment partials (with theta-shift corrections), then Newton on
    P(d) = a4m + k1 d + k2 d^2 + k3 d^3 using n = -d and fused STT Horner:
      P  = ((mk3*n + k2)*n - (-mk1))*n + a4m  with mk1 = -k1, mk3 = -k3
      P' = (q3*n + mq2)*n - mk1               with q3 = 3k3, mq2 = -2k2
      n <- n + P/P'
    Returns tau = theta - n."""
    Alu = mybir.AluOpType
    sm = pools["small"]
    a1p, a2p, a3p, a4p = pools[f"aparts{t}"]

    def tile(nm):
        return sm.tile([P, 1], F32, tag=f"{nm}_{t}", name=nm)

    a1s, a2s, a3s, a4s = tile("a1s"), tile("a2s"), tile("a3s"), tile("a4s")
    for acc, prt in zip((a1s, a2s, a3s, a4s), (a1p, a2p, a3p, a4p)):
        nc.vector.reduce_sum(out=acc, in_=prt, axis=mybir.AxisListType.X)

    a4m = tile("a4m")
    nc.vector.tensor_scalar(a4m, a4s, -1.0, None, op0=Alu.add)
    k2 = tile("k2")
    nc.vector.tensor_scalar(k2, a2s, 6.0, None, op0=Alu.mult)
    mq2 = tile("mq2")
    nc.vector.tensor_scalar(mq2, a2s, -12.0, None, op0=Alu.mult)
    # mk3 = 4*A1 = 4*a1s - 4D*theta ; q3 = -12*A1
    t4 = tile("t4")
    nc.vector.tensor_scalar(t4, a1s, 4.0, None, op0=Alu.mult)
    mk3 = tile("mk3")
    nc.vector.scalar_tensor_tensor(
        out=mk3, in0=theta, scalar=float(-4.0 * D), in1=t4,
        op0=Alu.mult, op1=Alu.add,
    )
    q3 = tile("q3")
    nc.vector.tensor_scalar(q3, mk3, -3.0, None, op0=Alu.mult)
    # mk1 = 4*A3 = 4*a3s - 4*theta*A2
    v = tile("v")
    nc.vector.tensor_mul(v, a2s, theta)
    t5 = tile("t5")
    nc.vector.tensor_scalar(t5, a3s, 4.0, None, op0=Alu.mult)
    mk1 = tile("mk1")
    nc.vector.scalar_tensor_tensor(
        out=mk1, in0=v, scalar=-4.0, in1=t5, op0=Alu.mult, op1=Alu.add,
    )

    n = tile("n")
    nc.vector.memset(n, 0.0)
    pv = tile("pv")
    ppv = tile("ppv")
    for _ in range(NEWTON_ITERS):
        nc.vector.scalar_tensor_tensor(
            out=pv, in0=n, scalar=mk3, in1=k2, op0=Alu.mult, op1=Alu.add)
        nc.vector.scalar_tensor_tensor(
            out=pv, in0=pv, scalar=n, in1=mk1, op0=Alu.mult, op1=Alu.add)
        nc.vector.scalar_tensor_tensor(
            out=pv, in0=pv, scalar=n, in1=a4m, op0=Alu.mult, op1=Alu.add)
        nc.vector.scalar_tensor_tensor(
            out=ppv, in0=n, scalar=q3, in1=mq2, op0=Alu.mult, op1=Alu.add)
        nc.vector.scalar_tensor_tensor(
            out=ppv, in0=ppv, scalar=n, in1=mk1, op0=Alu.mult, op1=Alu.subtract)
        nc.vector.reciprocal(ppv, ppv)
        nc.vector.scalar_tensor_tensor(
            out=n, in0=pv, scalar=ppv, in1=n, op0=Alu.mult, op1=Alu.add)

    tau = sm.tile([P, 1], F32, tag=f"tau{t}", name="tau")
    nc.vector.scalar_tensor_tensor(
        out=tau, in0=n, scalar=-1.0, in1=theta, op0=Alu.mult, op1=Alu.add)
    return tau


def _emit_output_sc(nc, pools, s, x16, tau, out_dram, row0, out0_dve):
    """One super-chunk of the output pass: yo (DVE 4x), zo (DVE 2x), then the
    two f32 out chunks: chunk 0 on DVE tensor_tensor when out0_dve (to
    offload ACT in the merged phase), chunk 1 on ACT Square."""
    Alu = mybir.AluOpType
    Act = mybir.ActivationFunctionType
    sl = slice(s * SC, (s + 1) * SC)
    yo = pools["y"].tile([P, SC], F16, tag="y", name="yo")
    nc.vector.tensor_scalar(yo, x16[:, sl], tau, 0.0, op0=Alu.subtract, op1=Alu.max)
    zo = pools["z"].tile([P, SC], F16, tag="z", name="zo")
    nc.vector.tensor_mul(zo, yo, yo)
    for k in range(SC // CHUNK):
        c = s * (SC // CHUNK) + k
        # loads are done by the time outputs flow: alternate across BOTH
        # staging pools for 4-deep rotation (store-sem latency hiding)
        n_o = pools["ldctr"][0]
        pools["ldctr"][0] += 1
        ostp = pools["ost"] if n_o % 2 == 0 else pools["ist"]
        ost = ostp.tile([P, CHUNK], F32, tag="st", name="ost")
        zsl = zo[:, k * CHUNK : (k + 1) * CHUNK]
        if k == 0 and out0_dve:
            nc.vector.tensor_mul(ost, zsl, zsl)
        else:
            nc.scalar.activation(ost, zsl, Act.Square)
        n_st = pools["stctr"][0]
        pools["stctr"][0] += 1
        stq = nc.sync if n_st % 2 == 0 else nc.scalar
        stq.dma_start(
            out=out_dram[row0 : row0 + P, c * CHUNK : (c + 1) * CHUNK], in_=ost
        )


def _core_program(nc, tc, x_dram, out_dram):
    import contextlib

    with contextlib.ExitStack() as stack:
        pools = {}
        for nm, bufs in (("ist", 2), ("ost", 2), ("x16p", 2), ("lmp", 2),
                         ("m1", 1), ("m2", 1), ("y", 2), ("z", 2),
                         ("bis", 1), ("small", 1)):
            pools[nm] = stack.enter_context(tc.tile_pool(name=nm, bufs=bufs))
        pools["ldctr"] = [0]
        pools["stctr"] = [0]
        sm = pools["small"]

        def tile_pools(t):
            d = dict(pools)
            d[f"aparts{t}"] = tuple(
                sm.tile([P, N_SC], F32, tag=f"a{m}p{t}", name=f"a{m}p{t}")
                for m in range(1, 5)
            )
            return d

        row0 = [t * P for t in range(N_ROW_TILES)]
        x16 = [pools["x16p"].tile([P, D], F16, tag="x16", name="x16")
               for _ in range(2)]
        tp = [tile_pools(0), tile_pools(1)]

        # ---- phase A0: tile 0 loads + tree (m1 on Pool) -------------------
        tp[0]["lmax_t"] = pools["lmp"].tile([P, LM_W], F16, tag="lmax",
                                            name="lmax")
        for c in range(N_CHUNKS):
            _emit_load_chunk(nc, tp[0], x16[0], c, x_dram, row0[0],
                             m1_pool=True)

        # ---- phase B0: tile 0 bracket + bisect ----------------------------
        # high_priority: the bracket gates everything downstream; without it
        # the Tile scheduler interleaves tile-1 conv ops (whose loads land
        # ~35us later) ahead of the reduce in the DVE queue.
        with tc.high_priority():
            theta0, ntheta0 = _emit_bracket(nc, tp[0], 0)
        # scale constant written AFTER tile 0's bracket: tile 1's conversions
        # read it, which forces the scheduler's DAG to place them behind the
        # bracket in the DVE queue (otherwise it interleaves them ahead of
        # the reduce and the bracket stalls on tile 1's late loads)
        qscale = pools["small"].tile([P, 1], F32, tag="qscale", name="qscale")
        # derive the constant from cmax so the dependency is real: 0*cmax+0.25
        nc.vector.tensor_scalar(
            qscale, tp[0]["cmax_tile0"][:, 0:1],
            0.0, 0.25, op0=mybir.AluOpType.mult, op1=mybir.AluOpType.add)
        tp[1]["qscale"] = qscale

        # ---- phase C0: tile 0 moments + tile 1 loads (m1 on DVE) ----------
        tp[1]["lmax_t"] = pools["lmp"].tile([P, LM_W], F16, tag="lmax",
                                            name="lmax")
        tp[1]["qflip"] = True
        for s in range(N_SC):
            _emit_moment_sc(nc, tp[0], 0, s, x16[0], theta0, ntheta0)
            with tc.high_priority(offset=-2000):
                for k in range(2):
                    _emit_load_chunk(nc, tp[1], x16[1], 2 * s + k, x_dram,
                                     row0[1], m1_pool=False)

        # ---- phase B1: tile 1 bracket + bisect (t1 data lands first) ------
        with tc.high_priority(offset=200):
            theta1, ntheta1 = _emit_bracket(nc, tp[1], 1)
        for k in range(2):
            _emit_moment_sc(nc, tp[1], 1, k, x16[1], theta1, ntheta1)

        # ---- phase D0: tile 0 Newton --------------------------------------
        with tc.high_priority(offset=200):
            tau0 = _emit_newton(nc, tp[0], 0, theta0)

        # ---- merged: rest of tile 1 moments + tile 0 output ---------------
        for k in range(N_SC):
            if k + 2 < N_SC:
                _emit_moment_sc(nc, tp[1], 1, k + 2, x16[1], theta1, ntheta1)
            _emit_output_sc(nc, tp[0], k, x16[0], tau0, out_dram,
                            row0[0], out0_dve=True)

        # ---- phase D1: tile 1 Newton --------------------------------------
        tau1 = _emit_newton(nc, tp[1], 1, theta1)

        # ---- phase E1: tile 1 output --------------------------------------
        for s in range(N_SC):
            _emit_output_sc(nc, tp[1], s, x16[1], tau1, out_dram, row0[1],
                            out0_dve=True)


def build_bass():
    from concourse import bacc

    nc = bacc.Bacc(None, target_bir_lowering=False)
    x_dram = nc.dram_tensor("x", [ROWS_PER_CORE, D], F32, kind="ExternalInput")
    out_dram = nc.dram_tensor("out", [ROWS_PER_CORE, D], F32, kind="ExternalOutput")
    with TileContext(nc) as tc:
        _core_program(nc, tc, x_dram, out_dram)
    nc.compile()
    return nc


_NC_CACHE = None


def kernel(input: np.ndarray) -> np.ndarray:
    global _NC_CACHE
    from concourse.bass_utils import run_bass_kernel_spmd

    x = np.ascontiguousarray(input, dtype=np.float32)
    assert x.shape == (ROWS_PER_CORE * N_CORES, D)

    if _NC_CACHE is None:
        _NC_CACHE = build_bass()
    nc = _NC_CACHE

    in_maps = [
        {"x": x[i * ROWS_PER_CORE : (i + 1) * ROWS_PER_CORE]} for i in range(N_CORES)
    ]
    res = run_bass_kernel_spmd(nc, in_maps, core_ids=list(range(N_CORES)))
    return np.concatenate([r["out"] for r in res.results], axis=0)
